# revision 8
# baseline (speedup 1.0000x reference)
"""AttentionWithPairBias Trainium2 kernel, 8-way sequence-parallel over query rows.

Strategy:
  - Each of the 8 cores owns 96 of the 768 query rows i.
  - The dominant work is the pair-bias reduction: pair [768,768,128] is
    host-transposed per core to [z=128, ij=96*768] so the z-contraction maps
    onto the TensorE partition axis. LayerNorm over z is algebraically folded:
        LN(z) @ (gz*Wb)  =  rsig_ij * (z @ W'')        (+ const_h, softmax-invariant)
    with W'' = gz*Wb - colsum(gz*Wb)/128.  mu and E[z^2] come out of the same
    matmuls via extra ones/128 columns; the squared stream is produced on
    ScalarE.  Four i-rows are packed per PSUM bank (partition offsets 0/32/64/96
    via zero-padded stationary operands) so the PSUM->SBUF copy runs with full
    partition utilization.  Per-(i,j) rsig is applied after a partition-remap
    SBUF->SBUF DMA puts the bias into [i, h, j] layout.
  - q/k/v/gate projections, attention, softmax (no max-subtraction: logits are
    O(6)), AV, and the output projection run per-core on its 96 rows.
  - All matmuls use float32r (full-rate PE, ~1e-3 rel precision).
"""
import sys

sys.path.insert(0, "/opt/trn_rl_repo")

import numpy as np

import concourse.bacc as bacc
import concourse.tile as tile
from concourse import mybir
from concourse.bass_utils import run_bass_kernel_spmd
from concourse.masks import make_identity
from contextlib import ExitStack

F32 = mybir.dt.float32
F32R = mybir.dt.float32r

L = 768
CS = 384
CZ = 128
H = 8
HD = 48
HP = 64          # padded head stride in permuted c2 layout
CP = H * HP      # 512, padded c2 size for q/k/v
NCORES = 8
LC = L // NCORES  # 96 rows per core
EPS = 1e-5
NQUAD = LC // 4   # 24 quads of 4 i-rows
JH = L // 2       # 384, half of j


def build():
    nc = bacc.Bacc("TRN2", target_bir_lowering=False, debug=False, num_devices=NCORES)

    pairT_d = nc.declare_dram_parameter("pairT", [CZ, LC * L], F32R, isOutput=False)
    sing_d = nc.declare_dram_parameter("sing", [L, CS], F32, isOutput=False)
    sown_d = nc.declare_dram_parameter("sown", [LC, CS], F32, isOutput=False)
    wraw_d = nc.declare_dram_parameter("wraw", [CZ, 4, 106], F32R, isOutput=False)
    wsq_d = nc.declare_dram_parameter("wsq", [CZ, 4, 106], F32R, isOutput=False)
    wqt_d = nc.declare_dram_parameter("wqt", [CS, CP], F32R, isOutput=False)
    wkt_d = nc.declare_dram_parameter("wkt", [CS, CP], F32R, isOutput=False)
    wvt_d = nc.declare_dram_parameter("wvt", [CS, CP], F32R, isOutput=False)
    wgt_d = nc.declare_dram_parameter("wgt", [CS, CS], F32R, isOutput=False)
    wot_d = nc.declare_dram_parameter("wot", [CS, CS], F32R, isOutput=False)
    qb_d = nc.declare_dram_parameter("qb", [CP, 1], F32, isOutput=False)
    kb_d = nc.declare_dram_parameter("kb", [CP, 1], F32, isOutput=False)
    vb_d = nc.declare_dram_parameter("vb", [CP], F32, isOutput=False)
    gb_d = nc.declare_dram_parameter("gb", [CS], F32, isOutput=False)
    bo_d = nc.declare_dram_parameter("bo", [CS], F32, isOutput=False)
    ident_d = nc.declare_dram_parameter("ident", [128, 128], F32R, isOutput=False)
    y_d = nc.declare_dram_parameter("y", [LC, CS], F32, isOutput=True)

    pairT3 = pairT_d[:].rearrange("z (i j) -> z i j", j=L)

    with tile.TileContext(nc) as tc, ExitStack() as ctx:
        singles = ctx.enter_context(tc.tile_pool(name="singles", bufs=1))
        persist = ctx.enter_context(tc.tile_pool(name="persist", bufs=1))
        stream = ctx.enter_context(tc.tile_pool(name="stream", bufs=3))
        small = ctx.enter_context(tc.tile_pool(name="small", bufs=4))
        pp_u = ctx.enter_context(tc.tile_pool(name="pp_u", bufs=3, space="PSUM"))
        pp_tp = ctx.enter_context(tc.tile_pool(name="pp_tp", bufs=2, space="PSUM"))
        pp_work = ctx.enter_context(tc.tile_pool(name="pp_work", bufs=2, space="PSUM"))

        # ---- constants / weights ----
        ident = singles.tile([128, 128], F32R)
        nc.sync.dma_start(out=ident, in_=ident_d[:])
        wraw_sb = singles.tile([CZ, 4, 106], F32R)
        wsq_sb = singles.tile([CZ, 4, 106], F32R)
        nc.sync.dma_start(out=wraw_sb, in_=wraw_d[:])
        nc.sync.dma_start(out=wsq_sb, in_=wsq_d[:])
        wqt_sb = singles.tile([128, 3, CP], F32R)
        wkt_sb = singles.tile([128, 3, CP], F32R)
        wvt_sb = singles.tile([128, 3, CP], F32R)
        wgt_sb = singles.tile([128, 3, CS], F32R)
        for b in range(3):
            nc.sync.dma_start(out=wqt_sb[:, b, :], in_=wqt_d[128 * b : 128 * (b + 1), :])
            nc.sync.dma_start(out=wkt_sb[:, b, :], in_=wkt_d[128 * b : 128 * (b + 1), :])
            nc.sync.dma_start(out=wvt_sb[:, b, :], in_=wvt_d[128 * b : 128 * (b + 1), :])
            nc.sync.dma_start(out=wgt_sb[:, b, :], in_=wgt_d[128 * b : 128 * (b + 1), :])
        wot_sb = singles.tile([HD, H, CS], F32R)
        nc.sync.dma_start(out=wot_sb, in_=wot_d[:].rearrange("(h d) n -> d h n", d=HD))
        qb_sb = singles.tile([128, 4, 1], F32)
        kb_sb = singles.tile([128, 4, 1], F32)
        nc.sync.dma_start(out=qb_sb, in_=qb_d[:].rearrange("(b p) o -> p b o", p=128))
        nc.sync.dma_start(out=kb_sb, in_=kb_d[:].rearrange("(b p) o -> p b o", p=128))

        def bcast_row(src, n, parts=128):
            t = singles.tile([parts, n], F32)
            import concourse.bass as bass
            ap = bass.AP(tensor=src.tensor, offset=src.offset, ap=[[0, parts]] + src.ap)
            nc.sync.dma_start(out=t, in_=ap)
            return t

        vb_bc = bcast_row(vb_d[:], CP)          # [128, 512]
        gb_bc = bcast_row(gb_d[:], CS)          # [128, 384]
        bo_bc = bcast_row(bo_d[:], CS)          # [128, 384]
        eps128 = singles.tile([128, 1], F32)
        nc.vector.memset(eps128, EPS)

        # ---- LayerNorm(single) ----
        s_sb = persist.tile([128, 6, CS], F32R)      # LN(single), i-major tiles
        so_sb = persist.tile([LC, CS], F32R)         # LN(single_own)
        sraw_sb = persist.tile([LC, CS], F32)        # raw single_own (residual)
        nc.sync.dma_start(out=sraw_sb, in_=sown_d[:])

        def layernorm(dst, src_ap, rows):
            x = stream.tile([128, CS], F32, tag="ln_x")
            nc.sync.dma_start(out=x[:rows], in_=src_ap)
            bn = small.tile([128, 6], F32, tag="ln_bn")
            nc.vector.bn_stats(out=bn[:rows], in_=x[:rows])
            mv = small.tile([128, 2], F32, tag="ln_mv")
            nc.vector.bn_aggr(out=mv[:rows], in_=bn[:rows])
            std = small.tile([128, 1], F32, tag="ln_std")
            nc.scalar.activation(out=std[:rows], in_=mv[:rows, 1:2],
                                 func=mybir.ActivationFunctionType.Sqrt,
                                 bias=eps128[:rows])
            rstd = small.tile([128, 1], F32, tag="ln_rstd")
            nc.vector.reciprocal(out=rstd[:rows], in_=std[:rows])
            nc.vector.tensor_scalar(out=dst, in0=x[:rows],
                                    scalar1=mv[:rows, 0:1], scalar2=rstd[:rows],
                                    op0=mybir.AluOpType.subtract,
                                    op1=mybir.AluOpType.mult)

        for t in range(6):
            layernorm(s_sb[:, t, :], sing_d[128 * t : 128 * (t + 1), :], 128)
        layernorm(so_sb[:], sown_d[:], LC)

        # ---- transposes: sT [c1, j] and sTo [c1, own-i] ----
        sT_sb = persist.tile([128, 3, L], F32R)
        for jb in range(6):
            for cb in range(3):
                pt = pp_tp.tile([128, 128], F32R, tag="tp")
                nc.tensor.transpose(pt, s_sb[:, jb, 128 * cb : 128 * (cb + 1)], ident)
                nc.vector.tensor_copy(out=sT_sb[:, cb, 128 * jb : 128 * (jb + 1)], in_=pt)
        sTo_sb = persist.tile([128, 3, LC], F32R)
        for cb in range(3):
            pt = pp_tp.tile([128, LC], F32R, tag="tp")
            nc.tensor.transpose(pt, so_sb[:, 128 * cb : 128 * (cb + 1)], ident[:LC, :LC])
            nc.vector.tensor_copy(out=sTo_sb[:, cb, :], in_=pt)

        # ---- projections ----
        qTo_sb = persist.tile([128, 4, LC], F32R)      # q^T (own rows), permuted heads
        for b in range(4):
            ps = pp_work.tile([128, 512], F32, tag="work")
            for kb in range(3):
                nc.tensor.matmul(ps[:, :LC], lhsT=wqt_sb[:, kb, 128 * b : 128 * (b + 1)],
                                 rhs=sTo_sb[:, kb, :], start=(kb == 0), stop=(kb == 2))
            nc.vector.tensor_scalar_add(out=qTo_sb[:, b, :], in0=ps[:, :LC],
                                        scalar1=qb_sb[:, b, :])

        kT_sb = persist.tile([128, 4, L], F32R)        # k^T (all rows), permuted heads
        for b in range(4):
            for jh in range(2):
                ps = pp_work.tile([128, 512], F32, tag="work")
                for kb in range(3):
                    nc.tensor.matmul(ps[:, :JH], lhsT=wkt_sb[:, kb, 128 * b : 128 * (b + 1)],
                                     rhs=sT_sb[:, kb, JH * jh : JH * (jh + 1)],
                                     start=(kb == 0), stop=(kb == 2))
                nc.vector.tensor_scalar_add(out=kT_sb[:, b, JH * jh : JH * (jh + 1)],
                                            in0=ps[:, :JH],
                                            scalar1=kb_sb[:, b, :])

        v_sb = persist.tile([128, 6, CP], F32R)        # v (all rows), [j, c2-perm]
        for jb in range(6):
            ps = pp_work.tile([128, 512], F32, tag="work")
            for kb in range(3):
                nc.tensor.matmul(ps, lhsT=sT_sb[:, kb, 128 * jb : 128 * (jb + 1)],
                                 rhs=wvt_sb[:, kb, :], start=(kb == 0), stop=(kb == 2))
            nc.vector.tensor_add(out=v_sb[:, jb, :], in0=ps, in1=vb_bc)

        gate_sb = persist.tile([LC, CS], F32)
        psg = pp_work.tile([128, 512], F32, tag="work")
        for kb in range(3):
            nc.tensor.matmul(psg[:LC, :CS], lhsT=sTo_sb[:, kb, :], rhs=wgt_sb[:, kb, :],
                             start=(kb == 0), stop=(kb == 2))
        gtmp = stream.tile([LC, CS], F32, tag="gtmp")
        nc.vector.tensor_add(out=gtmp, in0=psg[:LC, :CS], in1=gb_bc[:LC])
        nc.scalar.activation(out=gate_sb, in_=gtmp,
                             func=mybir.ActivationFunctionType.Sigmoid)

        # ---- pair-bias stream ----
        bias_hij = persist.tile([LC, H, L], F32)
        stats_sb = persist.tile([LC, 2, L], F32)
        for U in range(2 * NQUAD):
            Q, hf = U // 2, U % 2
            zt = stream.tile([CZ, 4, JH], F32R, tag="zt")
            nc.sync.dma_start(out=zt, in_=pairT3[:, 4 * Q : 4 * Q + 4, JH * hf : JH * (hf + 1)])
            sq = stream.tile([CZ, 4, JH], F32R, tag="sq")
            nc.scalar.activation(out=sq, in_=zt, func=mybir.ActivationFunctionType.Square)
            psu = pp_u.tile([128, JH], F32, tag="u")
            for q in range(4):
                nc.tensor.matmul(psu[0:106, :], lhsT=wraw_sb[:, q, :], rhs=zt[:, q, :],
                                 start=(q == 0), stop=False)
                nc.tensor.matmul(psu[0:106, :], lhsT=wsq_sb[:, q, :], rhs=sq[:, q, :],
                                 start=False, stop=(q == 3))
            staged = stream.tile([128, JH], F32, tag="staged")
            nc.vector.tensor_copy(out=staged, in_=psu)
            for q in range(4):
                i = 4 * Q + q
                nc.sync.dma_start(out=bias_hij[i : i + 1, :, JH * hf : JH * (hf + 1)],
                                  in_=staged[32 * q : 32 * q + 8, :])
                nc.sync.dma_start(out=stats_sb[i : i + 1, :, JH * hf : JH * (hf + 1)],
                                  in_=staged[32 * q + 8 : 32 * q + 10, :])

        # ---- rsig and bias scaling ----
        mu = stats_sb[:, 0, :]
        ex2 = stats_sb[:, 1, :]
        m2 = persist.tile([LC, L], F32)
        nc.vector.tensor_mul(out=m2, in0=mu, in1=mu)
        var = persist.tile([LC, L], F32)
        nc.vector.tensor_tensor(out=var, in0=ex2, in1=m2, op=mybir.AluOpType.subtract)
        std2 = persist.tile([LC, L], F32)
        nc.scalar.activation(out=std2, in_=var, func=mybir.ActivationFunctionType.Sqrt,
                             bias=eps128[:LC])
        rsig = persist.tile([LC, L], F32)
        nc.vector.reciprocal(out=rsig, in_=std2)
        for h in range(H):
            nc.vector.tensor_mul(out=bias_hij[:, h, :], in0=bias_hij[:, h, :], in1=rsig)

        # ---- attention per head ----
        outTo_sb = persist.tile([HD, H, LC], F32R)
        for h in range(H):
            blk, off = h // 2, HP * (h % 2)
            p_sb = stream.tile([LC, L], F32R, tag="p")
            rs = small.tile([LC, 2], F32, tag="rs")
            for jh in range(2):
                psl = pp_work.tile([128, 512], F32, tag="work")
                nc.tensor.matmul(psl[:LC, :JH],
                                 lhsT=qTo_sb[off : off + HD, blk, :],
                                 rhs=kT_sb[off : off + HD, blk, JH * jh : JH * (jh + 1)],
                                 start=True, stop=True)
                lgt = stream.tile([LC, JH], F32, tag="lgt")
                nc.vector.tensor_add(out=lgt, in0=psl[:LC, :JH],
                                     in1=bias_hij[:, h, JH * jh : JH * (jh + 1)])
                nc.scalar.activation(out=p_sb[:, JH * jh : JH * (jh + 1)], in_=lgt,
                                     func=mybir.ActivationFunctionType.Exp,
                                     accum_out=rs[:, jh : jh + 1])
            rsum = small.tile([LC, 1], F32, tag="rsum")
            nc.vector.tensor_add(out=rsum, in0=rs[:, 0:1], in1=rs[:, 1:2])
            rcp = small.tile([LC, 1], F32, tag="rcp")
            nc.vector.reciprocal(out=rcp, in_=rsum)
            nc.vector.tensor_scalar_mul(out=p_sb, in0=p_sb, scalar1=rcp)
            # transpose p -> pT, then AV
            psav = pp_work.tile([HD, LC], F32, tag="work")
            for jb in range(6):
                ptp = pp_tp.tile([128, LC], F32R, tag="tp")
                nc.tensor.transpose(ptp, p_sb[:, 128 * jb : 128 * (jb + 1)], ident[:LC, :LC])
                pT = stream.tile([128, LC], F32R, tag="pT")
                nc.vector.tensor_copy(out=pT, in_=ptp)
                nc.tensor.matmul(psav, lhsT=v_sb[:, jb, HP * h : HP * h + HD], rhs=pT,
                                 start=(jb == 0), stop=(jb == 5))
            nc.vector.tensor_copy(out=outTo_sb[:, h, :], in_=psav)

        # ---- output projection + gating + residual ----
        psy = pp_work.tile([128, 512], F32, tag="work")
        for h in range(H):
            nc.tensor.matmul(psy[:LC, :CS], lhsT=outTo_sb[:, h, :], rhs=wot_sb[:, h, :],
                             start=(h == 0), stop=(h == H - 1))
        fin = stream.tile([LC, CS], F32, tag="fin")
        nc.vector.tensor_add(out=fin, in0=psy[:LC, :CS], in1=bo_bc[:LC])
        nc.vector.tensor_mul(out=fin, in0=fin, in1=gate_sb)
        nc.vector.tensor_add(out=fin, in0=fin, in1=sraw_sb)
        nc.sync.dma_start(out=y_d[:], in_=fin)

    nc.compile()
    return nc


_NC = None


def _get_nc():
    global _NC
    if _NC is None:
        _NC = build()
    return _NC


def _host_prep(single, pair, g_s, b_s, g_z, b_z, Wq, Wk, Wv, Wb, Wo, bo, Wg, bg):
    f = np.float32
    single2d = np.asarray(single, f).reshape(L, CS)
    gs = np.asarray(g_s, f)
    bs = np.asarray(b_s, f)
    gz = np.asarray(g_z, f)

    # pair-bias weights with LN-mean folded in
    gW = gz[:, None] * np.asarray(Wb, f)                 # [CZ, H]
    Wpp = gW - gW.sum(0, keepdims=True) / CZ             # [CZ, H]
    wraw = np.zeros((CZ, 4, 106), f)
    wsq = np.zeros((CZ, 4, 106), f)
    for q in range(4):
        wraw[:, q, 32 * q : 32 * q + 8] = Wpp
        wraw[:, q, 32 * q + 8] = 1.0 / CZ
        wsq[:, q, 32 * q + 9] = 1.0 / CZ

    # head-permuted projection weights (c2' = 64h + d), g_s folded, scale folded into q
    def permute_heads(Wt):                               # Wt [c1, c2] -> [c1, CP]
        out = np.zeros((CS, CP), f)
        for h in range(H):
            out[:, HP * h : HP * h + HD] = Wt[:, HD * h : HD * (h + 1)]
        return out

    sc = 1.0 / np.sqrt(HD)
    WqT = (np.asarray(Wq, f) * sc).T * gs[:, None]       # [c1, c2]
    WkT = np.asarray(Wk, f).T * gs[:, None]
    WvT = np.asarray(Wv, f).T * gs[:, None]
    WgT = np.asarray(Wg, f).T * gs[:, None]
    WoT = np.asarray(Wo, f).T                            # [c1=(h,d), c2]
    wqt = permute_heads(WqT)
    wkt = permute_heads(WkT)
    wvt = permute_heads(WvT)

    def permute_vec(vec):                                # [CS] -> [CP]
        out = np.zeros(CP, f)
        for h in range(H):
            out[HP * h : HP * h + HD] = vec[HD * h : HD * (h + 1)]
        return out

    qb = permute_vec(bs @ (np.asarray(Wq, f) * sc).T)[:, None]
    kb = permute_vec(bs @ np.asarray(Wk, f).T)[:, None]
    vb = permute_vec(bs @ np.asarray(Wv, f).T)
    gb = (bs @ np.asarray(Wg, f).T + np.asarray(bg, f)).astype(f)
    bo_v = np.asarray(bo, f)

    pair4 = np.asarray(pair, f).reshape(L, L, CZ)
    shared = dict(sing=single2d, wraw=wraw, wsq=wsq, wqt=wqt, wkt=wkt, wvt=wvt,
                  wgt=np.ascontiguousarray(WgT), wot=np.ascontiguousarray(WoT),
                  qb=qb, kb=kb, vb=vb, gb=gb, bo=bo_v,
                  ident=np.eye(128, dtype=f))
    in_maps = []
    for c in range(NCORES):
        i0 = LC * c
        pT = np.ascontiguousarray(
            pair4[i0 : i0 + LC].reshape(LC * L, CZ).T)   # [CZ, LC*L]
        m = dict(shared)
        m["pairT"] = pT
        m["sown"] = np.ascontiguousarray(single2d[i0 : i0 + LC])
        in_maps.append(m)
    return in_maps


def kernel(**inputs) -> np.ndarray:
    nc = _get_nc()
    in_maps = _host_prep(**inputs)
    res = run_bass_kernel_spmd(nc, in_maps, list(range(NCORES)))
    out = np.empty((1, L, CS), np.float32)
    for c in range(NCORES):
        out[0, LC * c : LC * (c + 1)] = res.results[c]["y"]
    return out
